# revision 20
# baseline (speedup 1.0000x reference)
"""MoE router gate (DeepSeek-V3 style) on 8 Trainium2 NeuronCores.

Math (per token):
  logits = x @ w.T            [N=16384, E=256], D=7168, fp32
  scores = sigmoid(logits)
  s      = scores + bias
  group top-2 sums over 8 groups of 32 -> keep top-4 groups
  indices = top-8 of s within kept groups
  weights = renormalize(scores[indices]) * 2.5

Sharding: data-parallel over tokens (2048/core); w+bias replicated.

GEMM strategy ("Jmin" split, ~2.3 fp16-pass-equivalents vs 3 for the
classic hi/lo x2 split):
  x = xh + 2^-11 * x1        xh fp16, x1 fp8(e4m3)   (host-side split)
  w = wh + 2^-11 * wl        wh, wl fp16
  logits ~= xh@[wh|wl] (one fp16 N=512 pass, x-stationary)
          + 2^-11 * (x1@wh8)  (fp8 DoubleRow, w-stationary, [exp,tok]
                               orientation; cast fp16, PE-transposed back)
  Dropped terms are O(2^-15) on correction-sized operands; measured
  end-to-end index flip count on the real input: 6/16384 tokens.
"""

import os
import sys
import threading

sys.path.insert(0, "/opt/trn_rl_repo")

# bisection: "full" | "nodr" (plain fp8 matmuls) | "notp" (no transpose/merge)
VARIANT = os.environ.get("KERNEL_VARIANT", "full")

import numpy as np
import ml_dtypes

import concourse.bass as bass
import concourse.bacc as bacc
import concourse.mybir as mybir
import concourse.tile as tile
from concourse.bass_utils import run_bass_kernel_spmd

N_TOK = 16384
D = 7168
E = 256
N_CORES = 8
NSH = N_TOK // N_CORES          # tokens per core
TOK_TILE = 128
N_TILES = NSH // TOK_TILE       # 16
KC = 128                        # contraction chunk
N_KC = D // KC                  # 56
N_KP = N_KC // 2                # 28 chunk pairs (DoubleRow)
BLK = 512                       # corr token block
N_BLK = NSH // BLK              # 4
N_GROUPS = 8
GSIZE = E // N_GROUPS           # 32
TOPK = 8
ROUTE_SCALE = 2.5
SPLIT_SCALE = 2048.0            # 2^11
NEG_BIG = 1.0e30

# chunk groups for DMA-granular dependencies (all even so DoubleRow
# chunk pairs never straddle a group boundary)
GS = [2, 6] + [8] * 6
GOFF = [sum(GS[:i]) for i in range(len(GS))]
NG = len(GS)
C2G = []
for _gi, _n in enumerate(GS):
    C2G += [(_gi, _c) for _c in range(_n)]

# after which hi-tile to emit each corr block's matmuls. With ps1
# bufs=3, corr b may trail hi(4b) by up to 2 tiles before a ps1-buffer
# recycle would wait on a not-yet-issued corr (merge of tile 4b gates
# tile 4b+3's buffer).
CORR_AFTER = {3: 0, 7: 1, 11: 2, 12: 3}
# after which hi-tile to issue each x1 block's DMA (x1-b0 intentionally
# after the xh-b1 load so early hi tiles aren't starved behind it)
X1_PREFETCH = {2: 1, 6: 2, 10: 3}

_cached = {}


def _build_nc():
    fp16 = mybir.dt.float16
    fp8 = mybir.dt.float8e4
    f32 = mybir.dt.float32
    u32 = mybir.dt.uint32

    nc = bacc.Bacc(trn_type="TRN2", target_bir_lowering=False)

    xh_d = nc.dram_tensor("xh", [D, NSH], fp16, kind="ExternalInput")
    x1_d = nc.dram_tensor("x1", [D, NSH], fp8, kind="ExternalInput")
    # w packed [D, 512]: cols 0:256 = wh, 256:512 = wl (both fp16, wl scaled)
    w_d = nc.dram_tensor("w", [D, 2 * E], fp16, kind="ExternalInput")
    wh8_d = nc.dram_tensor("wh8", [D, E], fp8, kind="ExternalInput")
    bias_d = nc.dram_tensor("bias", [128, E], f32, kind="ExternalInput")
    ident_d = nc.dram_tensor("ident", [128, 128], fp16, kind="ExternalInput")
    wts_d = nc.dram_tensor("wts", [NSH, TOPK], f32, kind="ExternalOutput")
    idx_d = nc.dram_tensor("idx", [NSH, TOPK], mybir.dt.int32, kind="ExternalOutput")

    with tile.TileContext(nc) as tc:
        with (
            tc.tile_pool(name="wpool", bufs=1) as wpool,
            tc.tile_pool(name="xpool", bufs=2) as xpool,
            tc.tile_pool(name="x1pool", bufs=2) as x1pool,
            tc.tile_pool(name="spool", bufs=2) as spool,
            tc.tile_pool(name="tiny", bufs=2) as tiny,
            tc.tile_pool(name="sct", bufs=2) as sctp,
            tc.tile_pool(name="psum1", bufs=4, space="PSUM") as ps1pool,
            tc.tile_pool(name="psumc", bufs=1, space="PSUM") as pscpool,
            tc.tile_pool(name="psumt", bufs=2, space="PSUM") as pstpool,
        ):
            # --- resident weights / bias / identity ---
            wsb_g = []
            xh_cur = [None] * NG
            for g in range(NG):
                r0, r1 = GOFF[g] * 128, (GOFF[g] + GS[g]) * 128
                wg = wpool.tile([128, GS[g], 2 * E], fp16, tag=f"w{g}", bufs=1)
                nc.sync.dma_start(
                    wg[:, :, :],
                    w_d[r0:r1, :].rearrange("(c p) e -> p c e", p=128),
                )
                wsb_g.append(wg)
                xhg = xpool.tile([128, GS[g], 2 * TOK_TILE], fp16, tag=f"xh{g}", bufs=2)
                nc.sync.dma_start(
                    xhg[:, :, :],
                    xh_d[r0:r1, 0 : 2 * TOK_TILE].rearrange("(c p) n -> p c n", p=128),
                )
                xh_cur[g] = xhg
            wh8_sb = wpool.tile([128, N_KC, E], fp8, tag="wh8")
            nc.sync.dma_start(
                wh8_sb[:, :, :], wh8_d[:, :].rearrange("(c p) e -> p c e", p=128)
            )
            bias_sb = wpool.tile([128, E], f32, tag="bias")
            nc.scalar.dma_start(bias_sb[:, :], bias_d[:, :])
            ident_sb = wpool.tile([128, 128], fp16, tag="ident")
            nc.scalar.dma_start(ident_sb[:, :], ident_d[:, :])

            x1_blocks = [None] * N_BLK

            def load_x1_block(b):
                tiles = []
                for g in range(NG):
                    r0, r1 = GOFF[g] * 128, (GOFF[g] + GS[g]) * 128
                    xg = x1pool.tile([128, GS[g], BLK], fp8, tag=f"x1{g}", bufs=2)
                    nc.sync.dma_start(
                        xg[:, :, :],
                        x1_d[r0:r1, b * BLK : (b + 1) * BLK].rearrange(
                            "(c p) n -> p c n", p=128
                        ),
                    )
                    tiles.append(xg)
                x1_blocks[b] = tiles

            load_x1_block(0)

            sct_by_block = [None] * N_BLK

            def emit_corr(b):
                """fp8 DoubleRow corr pass for token block b -> scT fp16."""
                x1g = x1_blocks[b]
                psc = [
                    pscpool.tile(
                        [128, BLK], f32, tag=f"psc{h}", bufs=1, name=f"psc{h}"
                    )
                    for h in range(2)
                ]
                if VARIANT == "nodr":
                    for c in range(N_KC):
                        g, ci = C2G[c]
                        for h in range(2):
                            nc.tensor.matmul(
                                psc[h][:, :],
                                wh8_sb[:, c, h * 128 : (h + 1) * 128],
                                x1g[g][:, ci, :],
                                start=(c == 0),
                                stop=(c == N_KC - 1),
                            )
                else:
                    for kp in range(N_KP):
                        g, ci = C2G[2 * kp]
                        for h in range(2):
                            nc.tensor.matmul(
                                psc[h][:, :],
                                wh8_sb[:, 2 * kp : 2 * kp + 2, h * 128 : (h + 1) * 128],
                                x1g[g][:, ci : ci + 2, :],
                                start=(kp == 0),
                                stop=(kp == N_KP - 1),
                                perf_mode=mybir.MatmulPerfMode.DoubleRow,
                            )
                sct = sctp.tile([128, 2, BLK], fp16, tag="sct", bufs=2)
                for h in range(2):
                    nc.scalar.activation(
                        sct[:, h, :], psc[h][:, :],
                        mybir.ActivationFunctionType.Copy,
                    )
                sct_by_block[b] = sct

            ps1_by_tile = [None] * N_TILES

            def emit_routing(t):
                ts = t * TOK_TILE
                ps1 = ps1_by_tile[t]
                sct = sct_by_block[t // 4]
                # transpose corr slice for this tile: [exp,tok]->[tok,exp]
                bo = (t % 4) * TOK_TILE
                if VARIANT == "notp":
                    logits = spool.tile([128, E], f32, tag="logits")
                    nc.vector.scalar_tensor_tensor(
                        logits[:, :], ps1[:, E:], 1.0 / SPLIT_SCALE, ps1[:, 0:E],
                        op0=mybir.AluOpType.mult, op1=mybir.AluOpType.add,
                    )
                else:
                    pst = pstpool.tile([128, 1024], fp16, tag="pst", bufs=2)
                    for h in range(2):
                        nc.tensor.matmul(
                            pst[:, h * 128 : (h + 1) * 128],
                            sct[:, h, bo : bo + TOK_TILE],
                            ident_sb[:, :],
                            is_transpose=True,
                            start=(h == 0),
                            stop=(h == 1),
                        )

                    # logits = ps1[:, :E] + 2^-11*(ps1[:, E:] + corrT)
                    # (walrus NCC_IBVF027: at most one PSUM operand per op)
                    cT = spool.tile([128, E], f32, tag="cT")
                    nc.vector.tensor_copy(cT[:, :], pst[:, 0:256])
                    u = spool.tile([128, E], f32, tag="u")
                    nc.vector.tensor_add(u[:, :], cT[:, :], ps1[:, E:])
                    logits = spool.tile([128, E], f32, tag="logits")
                    nc.vector.scalar_tensor_tensor(
                        logits[:, :], u[:, :], 1.0 / SPLIT_SCALE, ps1[:, 0:E],
                        op0=mybir.AluOpType.mult, op1=mybir.AluOpType.add,
                    )

                # scores = sigmoid(logits); s = scores + bias
                scores = spool.tile([128, E], f32, tag="scores")
                nc.scalar.activation(
                    scores[:, :], logits[:, :], mybir.ActivationFunctionType.Sigmoid
                )
                s = spool.tile([128, E], f32, tag="s")
                nc.vector.tensor_add(s[:, :], scores[:, :], bias_sb[:, :])

                # group top-2 sums
                gtop = tiny.tile([128, N_GROUPS, 8], f32, tag="gtop")
                for g in range(N_GROUPS):
                    nc.vector.max(gtop[:, g, :], s[:, g * GSIZE : (g + 1) * GSIZE])
                gs = tiny.tile([128, N_GROUPS], f32, tag="gs")
                nc.vector.tensor_add(gs[:, :], gtop[:, :, 0], gtop[:, :, 1])

                gsort = tiny.tile([128, 8], f32, tag="gsort")
                nc.vector.max(gsort[:, :], gs[:, :])
                # amask = 0 for kept groups (gs >= 4th group score), -BIG else
                amask = tiny.tile([128, N_GROUPS], f32, tag="amask")
                nc.vector.tensor_scalar(
                    amask[:, :], gs[:, :], gsort[:, 3:4], -NEG_BIG,
                    op0=mybir.AluOpType.is_lt, op1=mybir.AluOpType.mult,
                )

                smask = spool.tile([128, N_GROUPS, GSIZE], f32, tag="smask")
                nc.vector.tensor_tensor(
                    smask[:, :, :],
                    s[:, :].rearrange("p (g e) -> p g e", g=N_GROUPS),
                    amask[:, :].unsqueeze(-1).broadcast_to([128, N_GROUPS, GSIZE]),
                    op=mybir.AluOpType.add,
                )

                smask2 = smask[:, :, :].rearrange("p g e -> p (g e)")
                top8v = tiny.tile([128, TOPK], f32, tag="top8v")
                nc.vector.max(top8v[:, :], smask2)
                top8i = tiny.tile([128, TOPK], u32, tag="top8i")
                nc.vector.max_index(top8i[:, :], top8v[:, :], smask2)

                # extract scores at selected positions, aligned to top8v order
                wsel = tiny.tile([128, TOPK], f32, tag="wsel")
                scratch = spool.tile([128, E], f32, tag="scratch")
                for j in range(TOPK):
                    nc.vector.scalar_tensor_tensor(
                        scratch[:, :], smask2, top8v[:, j : j + 1], scores[:, :],
                        op0=mybir.AluOpType.is_equal, op1=mybir.AluOpType.mult,
                        accum_out=wsel[:, j : j + 1],
                    )

                ssum = tiny.tile([128, 1], f32, tag="ssum")
                nc.vector.reduce_sum(ssum[:, :], wsel[:, :], axis=mybir.AxisListType.X)
                rec = tiny.tile([128, 1], f32, tag="rec")
                nc.vector.reciprocal(rec[:, :], ssum[:, :])
                wout = tiny.tile([128, TOPK], f32, tag="wout")
                nc.vector.tensor_scalar(
                    wout[:, :], wsel[:, :], rec[:, 0:1], ROUTE_SCALE,
                    op0=mybir.AluOpType.mult, op1=mybir.AluOpType.mult,
                )

                nc.sync.dma_start(wts_d[ts : ts + TOK_TILE, :], wout[:, :])
                nc.sync.dma_start(
                    idx_d[ts : ts + TOK_TILE, :],
                    top8i[:, :].bitcast(mybir.dt.int32),
                )

            pending = []
            for t in range(N_TILES):
                sub = t % 2
                if sub == 0 and t > 0:
                    bs = t * TOK_TILE
                    for g in range(NG):
                        r0, r1 = GOFF[g] * 128, (GOFF[g] + GS[g]) * 128
                        xhg = xpool.tile(
                            [128, GS[g], 2 * TOK_TILE], fp16, tag=f"xh{g}", bufs=2
                        )
                        nc.sync.dma_start(
                            xhg[:, :, :],
                            xh_d[r0:r1, bs : bs + 2 * TOK_TILE].rearrange(
                                "(c p) n -> p c n", p=128
                            ),
                        )
                        xh_cur[g] = xhg
                if t in X1_PREFETCH:
                    load_x1_block(X1_PREFETCH[t])

                tsl = slice(sub * TOK_TILE, (sub + 1) * TOK_TILE)
                ps1 = ps1pool.tile([128, 2 * E], f32, tag="ps1")
                ps1_by_tile[t] = ps1
                for c in range(N_KC):
                    g, ci = C2G[c]
                    nc.tensor.matmul(
                        ps1[:, :],
                        xh_cur[g][:, ci, tsl],
                        wsb_g[g][:, ci, :],
                        start=(c == 0),
                        stop=(c == N_KC - 1),
                    )

                if t in CORR_AFTER:
                    emit_corr(CORR_AFTER[t])

                pending.append(t)
                while pending and sct_by_block[pending[0] // 4] is not None:
                    emit_routing(pending.pop(0))
    nc.finalize()
    return nc


def _host_prep(x, weight, bias):
    """fp16 hi + fp8 lo split, d-major transpose, per-core shards."""
    x = np.asarray(x, dtype=np.float32)
    weight = np.asarray(weight, dtype=np.float32)
    bias = np.asarray(bias, dtype=np.float32)
    f8 = ml_dtypes.float8_e4m3

    wh = weight.astype(np.float16)
    wl = ((weight - wh.astype(np.float32)) * SPLIT_SCALE).astype(np.float16)
    w_packed = np.empty((D, 2 * E), dtype=np.float16)
    w_packed[:, :E] = wh.T
    w_packed[:, E:] = wl.T
    wh8 = np.ascontiguousarray(wh.T).astype(f8)
    bias_rep = np.ascontiguousarray(np.broadcast_to(bias[None, :], (128, E)))
    ident = np.eye(128, dtype=np.float16)

    in_maps = [None] * N_CORES

    def prep_core(c):
        xs = x[c * NSH : (c + 1) * NSH, :]
        xh = xs.astype(np.float16)
        x1 = ((xs - xh.astype(np.float32)) * SPLIT_SCALE).astype(f8)
        in_maps[c] = {
            "xh": np.ascontiguousarray(xh.T),
            "x1": np.ascontiguousarray(x1.T),
            "w": w_packed,
            "wh8": wh8,
            "bias": bias_rep,
            "ident": ident,
        }

    threads = [threading.Thread(target=prep_core, args=(c,)) for c in range(N_CORES)]
    for th in threads:
        th.start()
    for th in threads:
        th.join()
    return in_maps


def kernel(x, weight, bias, _trace=False):
    if "nc" not in _cached:
        _cached["nc"] = _build_nc()
    nc = _cached["nc"]
    in_maps = _host_prep(x, weight, bias)
    res = run_bass_kernel_spmd(
        nc, in_maps, core_ids=list(range(N_CORES)), trace=_trace
    )
    _cached["last_result"] = res
    wts = np.concatenate([r["wts"] for r in res.results], axis=0)
    idx = np.concatenate([r["idx"] for r in res.results], axis=0)
    return wts, idx


# revision 22
# speedup vs baseline: 1.0087x; 1.0087x over previous
"""MoE router gate (DeepSeek-V3 style) on 8 Trainium2 NeuronCores.

Math (per token):
  logits = x @ w.T            [N=16384, E=256], D=7168, fp32
  scores = sigmoid(logits)
  s      = scores + bias
  group top-2 sums over 8 groups of 32 -> keep top-4 groups
  indices = top-8 of s within kept groups
  weights = renormalize(scores[indices]) * 2.5

Sharding: data-parallel over tokens (2048/core); w+bias replicated.

GEMM strategy ("Jmin" split, ~2.3 fp16-pass-equivalents vs 3 for the
classic hi/lo x2 split):
  x = xh + 2^-11 * x1        xh fp16, x1 fp8(e4m3)   (host-side split)
  w = wh + 2^-11 * wl        wh, wl fp16
  logits ~= xh@[wh|wl] (one fp16 N=512 pass, x-stationary)
          + 2^-11 * (x1@wh8)  (fp8 DoubleRow, w-stationary, [exp,tok]
                               orientation; cast fp16, PE-transposed back)
  Dropped terms are O(2^-15) on correction-sized operands; measured
  end-to-end index flip count on the real input: 6/16384 tokens.
"""

import os
import sys
import threading

sys.path.insert(0, "/opt/trn_rl_repo")

# bisection: "full" | "nodr" (plain fp8 matmuls) | "notp" (no transpose/merge)
VARIANT = os.environ.get("KERNEL_VARIANT", "full")

import numpy as np
import ml_dtypes

import concourse.bass as bass
import concourse.bacc as bacc
import concourse.mybir as mybir
import concourse.tile as tile
from concourse.bass_utils import run_bass_kernel_spmd

N_TOK = 16384
D = 7168
E = 256
N_CORES = 8
NSH = N_TOK // N_CORES          # tokens per core
TOK_TILE = 128
N_TILES = NSH // TOK_TILE       # 16
KC = 128                        # contraction chunk
N_KC = D // KC                  # 56
N_KP = N_KC // 2                # 28 chunk pairs (DoubleRow)
BLK = 512                       # corr token block
N_BLK = NSH // BLK              # 4
N_GROUPS = 8
GSIZE = E // N_GROUPS           # 32
TOPK = 8
ROUTE_SCALE = 2.5
SPLIT_SCALE = 2048.0            # 2^11
NEG_BIG = 1.0e30

# chunk groups for DMA-granular dependencies (all even so DoubleRow
# chunk pairs never straddle a group boundary)
GS = [2, 6] + [8] * 6
GOFF = [sum(GS[:i]) for i in range(len(GS))]
NG = len(GS)
C2G = []
for _gi, _n in enumerate(GS):
    C2G += [(_gi, _c) for _c in range(_n)]

# corr block b's 56 DoubleRow instructions are interleaved 1:1 into the
# hi chunk loop of tile CORR_INSIDE^-1[b]: the corr LDWEIGHTS (213 ns)
# hides behind the preceding hi matmul (216 ns) via the PE's LDW
# pull-ahead, instead of serializing LDW-bound at 213 ns/instr. Block 3
# sits at tile 12 so tiles 13-15 route inline (tail unchanged); with ps1
# bufs=4 the block-b merges land just in time for tile 4b+4's buffer.
CORR_INSIDE = {3: 0, 7: 1, 11: 2, 12: 3}
# after which hi-tile to issue each x1 block's DMA (x1-b0 intentionally
# after the xh-b1 load so early hi tiles aren't starved behind it)
X1_PREFETCH = {2: 1, 6: 2, 10: 3}

_cached = {}


def _build_nc():
    fp16 = mybir.dt.float16
    fp8 = mybir.dt.float8e4
    f32 = mybir.dt.float32
    u32 = mybir.dt.uint32

    nc = bacc.Bacc(trn_type="TRN2", target_bir_lowering=False)

    xh_d = nc.dram_tensor("xh", [D, NSH], fp16, kind="ExternalInput")
    x1_d = nc.dram_tensor("x1", [D, NSH], fp8, kind="ExternalInput")
    # w packed [D, 512]: cols 0:256 = wh, 256:512 = wl (both fp16, wl scaled)
    w_d = nc.dram_tensor("w", [D, 2 * E], fp16, kind="ExternalInput")
    wh8_d = nc.dram_tensor("wh8", [D, E], fp8, kind="ExternalInput")
    bias_d = nc.dram_tensor("bias", [128, E], f32, kind="ExternalInput")
    ident_d = nc.dram_tensor("ident", [128, 128], fp16, kind="ExternalInput")
    wts_d = nc.dram_tensor("wts", [NSH, TOPK], f32, kind="ExternalOutput")
    idx_d = nc.dram_tensor("idx", [NSH, TOPK], mybir.dt.int32, kind="ExternalOutput")

    with tile.TileContext(nc) as tc:
        with (
            tc.tile_pool(name="wpool", bufs=1) as wpool,
            tc.tile_pool(name="xpool", bufs=2) as xpool,
            tc.tile_pool(name="x1pool", bufs=2) as x1pool,
            tc.tile_pool(name="spool", bufs=2) as spool,
            tc.tile_pool(name="tiny", bufs=2) as tiny,
            tc.tile_pool(name="sct", bufs=2) as sctp,
            tc.tile_pool(name="psum1", bufs=4, space="PSUM") as ps1pool,
            tc.tile_pool(name="psumc", bufs=1, space="PSUM") as pscpool,
            tc.tile_pool(name="psumt", bufs=2, space="PSUM") as pstpool,
        ):
            # --- resident weights / bias / identity ---
            wsb_g = []
            xh_cur = [None] * NG
            for g in range(NG):
                r0, r1 = GOFF[g] * 128, (GOFF[g] + GS[g]) * 128
                wg = wpool.tile([128, GS[g], 2 * E], fp16, tag=f"w{g}", bufs=1)
                nc.sync.dma_start(
                    wg[:, :, :],
                    w_d[r0:r1, :].rearrange("(c p) e -> p c e", p=128),
                )
                wsb_g.append(wg)
                xhg = xpool.tile([128, GS[g], 2 * TOK_TILE], fp16, tag=f"xh{g}", bufs=2)
                nc.sync.dma_start(
                    xhg[:, :, :],
                    xh_d[r0:r1, 0 : 2 * TOK_TILE].rearrange("(c p) n -> p c n", p=128),
                )
                xh_cur[g] = xhg
            wh8_sb = wpool.tile([128, N_KC, E], fp8, tag="wh8")
            nc.sync.dma_start(
                wh8_sb[:, :, :], wh8_d[:, :].rearrange("(c p) e -> p c e", p=128)
            )
            bias_sb = wpool.tile([128, E], f32, tag="bias")
            nc.scalar.dma_start(bias_sb[:, :], bias_d[:, :])
            ident_sb = wpool.tile([128, 128], fp16, tag="ident")
            nc.scalar.dma_start(ident_sb[:, :], ident_d[:, :])

            x1_blocks = [None] * N_BLK

            def load_x1_block(b):
                tiles = []
                for g in range(NG):
                    r0, r1 = GOFF[g] * 128, (GOFF[g] + GS[g]) * 128
                    xg = x1pool.tile([128, GS[g], BLK], fp8, tag=f"x1{g}", bufs=2)
                    nc.sync.dma_start(
                        xg[:, :, :],
                        x1_d[r0:r1, b * BLK : (b + 1) * BLK].rearrange(
                            "(c p) n -> p c n", p=128
                        ),
                    )
                    tiles.append(xg)
                x1_blocks[b] = tiles

            load_x1_block(0)

            sct_by_block = [None] * N_BLK

            def make_corr_psc():
                return [
                    pscpool.tile(
                        [128, BLK], f32, tag=f"psc{h}", bufs=1, name=f"psc{h}"
                    )
                    for h in range(2)
                ]

            def emit_corr_instr(b, psc, c):
                """c-th of 56 DoubleRow corr instrs for block b."""
                kp, h = c // 2, c % 2
                g, ci = C2G[2 * kp]
                nc.tensor.matmul(
                    psc[h][:, :],
                    wh8_sb[:, 2 * kp : 2 * kp + 2, h * 128 : (h + 1) * 128],
                    x1_blocks[b][g][:, ci : ci + 2, :],
                    start=(kp == 0),
                    stop=(kp == N_KP - 1),
                    perf_mode=mybir.MatmulPerfMode.DoubleRow,
                )

            def finish_corr(b, psc):
                sct = sctp.tile([128, 2, BLK], fp16, tag="sct", bufs=2)
                for h in range(2):
                    nc.scalar.activation(
                        sct[:, h, :], psc[h][:, :],
                        mybir.ActivationFunctionType.Copy,
                    )
                sct_by_block[b] = sct

            ps1_by_tile = [None] * N_TILES

            def emit_routing(t):
                ts = t * TOK_TILE
                ps1 = ps1_by_tile[t]
                sct = sct_by_block[t // 4]
                # transpose corr slice for this tile: [exp,tok]->[tok,exp]
                bo = (t % 4) * TOK_TILE
                if VARIANT == "notp":
                    logits = spool.tile([128, E], f32, tag="logits")
                    nc.vector.scalar_tensor_tensor(
                        logits[:, :], ps1[:, E:], 1.0 / SPLIT_SCALE, ps1[:, 0:E],
                        op0=mybir.AluOpType.mult, op1=mybir.AluOpType.add,
                    )
                else:
                    pst = pstpool.tile([128, 1024], fp16, tag="pst", bufs=2)
                    for h in range(2):
                        nc.tensor.matmul(
                            pst[:, h * 128 : (h + 1) * 128],
                            sct[:, h, bo : bo + TOK_TILE],
                            ident_sb[:, :],
                            is_transpose=True,
                            start=(h == 0),
                            stop=(h == 1),
                        )

                    # logits = ps1[:, :E] + 2^-11*(ps1[:, E:] + corrT)
                    # (walrus NCC_IBVF027: at most one PSUM operand per op)
                    cT = spool.tile([128, E], f32, tag="cT")
                    nc.scalar.activation(
                        cT[:, :], pst[:, 0:256], mybir.ActivationFunctionType.Copy
                    )
                    u = spool.tile([128, E], f32, tag="u")
                    nc.vector.tensor_add(u[:, :], cT[:, :], ps1[:, E:])
                    logits = spool.tile([128, E], f32, tag="logits")
                    nc.vector.scalar_tensor_tensor(
                        logits[:, :], u[:, :], 1.0 / SPLIT_SCALE, ps1[:, 0:E],
                        op0=mybir.AluOpType.mult, op1=mybir.AluOpType.add,
                    )

                # scores = sigmoid(logits); s = scores + bias
                scores = spool.tile([128, E], f32, tag="scores")
                nc.scalar.activation(
                    scores[:, :], logits[:, :], mybir.ActivationFunctionType.Sigmoid
                )
                s = spool.tile([128, E], f32, tag="s")
                nc.vector.tensor_add(s[:, :], scores[:, :], bias_sb[:, :])

                # group top-2 sums
                gtop = tiny.tile([128, N_GROUPS, 8], f32, tag="gtop")
                for g in range(N_GROUPS):
                    nc.vector.max(gtop[:, g, :], s[:, g * GSIZE : (g + 1) * GSIZE])
                gs = tiny.tile([128, N_GROUPS], f32, tag="gs")
                nc.vector.tensor_add(gs[:, :], gtop[:, :, 0], gtop[:, :, 1])

                gsort = tiny.tile([128, 8], f32, tag="gsort")
                nc.vector.max(gsort[:, :], gs[:, :])
                # amask = 0 for kept groups (gs >= 4th group score), -BIG else
                amask = tiny.tile([128, N_GROUPS], f32, tag="amask")
                nc.vector.tensor_scalar(
                    amask[:, :], gs[:, :], gsort[:, 3:4], -NEG_BIG,
                    op0=mybir.AluOpType.is_lt, op1=mybir.AluOpType.mult,
                )

                smask = spool.tile([128, N_GROUPS, GSIZE], f32, tag="smask")
                nc.vector.tensor_tensor(
                    smask[:, :, :],
                    s[:, :].rearrange("p (g e) -> p g e", g=N_GROUPS),
                    amask[:, :].unsqueeze(-1).broadcast_to([128, N_GROUPS, GSIZE]),
                    op=mybir.AluOpType.add,
                )

                smask2 = smask[:, :, :].rearrange("p g e -> p (g e)")
                top8v = tiny.tile([128, TOPK], f32, tag="top8v")
                nc.vector.max(top8v[:, :], smask2)
                top8i = tiny.tile([128, TOPK], u32, tag="top8i")
                nc.vector.max_index(top8i[:, :], top8v[:, :], smask2)

                # extract scores at selected positions, aligned to top8v order
                wsel = tiny.tile([128, TOPK], f32, tag="wsel")
                scratch = spool.tile([128, E], f32, tag="scratch")
                for j in range(TOPK):
                    nc.vector.scalar_tensor_tensor(
                        scratch[:, :], smask2, top8v[:, j : j + 1], scores[:, :],
                        op0=mybir.AluOpType.is_equal, op1=mybir.AluOpType.mult,
                        accum_out=wsel[:, j : j + 1],
                    )

                ssum = tiny.tile([128, 1], f32, tag="ssum")
                nc.vector.reduce_sum(ssum[:, :], wsel[:, :], axis=mybir.AxisListType.X)
                rec = tiny.tile([128, 1], f32, tag="rec")
                nc.vector.reciprocal(rec[:, :], ssum[:, :])
                wout = tiny.tile([128, TOPK], f32, tag="wout")
                nc.vector.tensor_scalar(
                    wout[:, :], wsel[:, :], rec[:, 0:1], ROUTE_SCALE,
                    op0=mybir.AluOpType.mult, op1=mybir.AluOpType.mult,
                )

                nc.sync.dma_start(wts_d[ts : ts + TOK_TILE, :], wout[:, :])
                nc.sync.dma_start(
                    idx_d[ts : ts + TOK_TILE, :],
                    top8i[:, :].bitcast(mybir.dt.int32),
                )

            pending = []
            for t in range(N_TILES):
                sub = t % 2
                if sub == 0 and t > 0:
                    bs = t * TOK_TILE
                    for g in range(NG):
                        r0, r1 = GOFF[g] * 128, (GOFF[g] + GS[g]) * 128
                        xhg = xpool.tile(
                            [128, GS[g], 2 * TOK_TILE], fp16, tag=f"xh{g}", bufs=2
                        )
                        nc.sync.dma_start(
                            xhg[:, :, :],
                            xh_d[r0:r1, bs : bs + 2 * TOK_TILE].rearrange(
                                "(c p) n -> p c n", p=128
                            ),
                        )
                        xh_cur[g] = xhg
                if t in X1_PREFETCH:
                    load_x1_block(X1_PREFETCH[t])

                tsl = slice(sub * TOK_TILE, (sub + 1) * TOK_TILE)
                ps1 = ps1pool.tile([128, 2 * E], f32, tag="ps1")
                ps1_by_tile[t] = ps1
                cb = CORR_INSIDE.get(t)
                psc = make_corr_psc() if cb is not None else None
                for c in range(N_KC):
                    g, ci = C2G[c]
                    nc.tensor.matmul(
                        ps1[:, :],
                        xh_cur[g][:, ci, tsl],
                        wsb_g[g][:, ci, :],
                        start=(c == 0),
                        stop=(c == N_KC - 1),
                    )
                    if cb is not None:
                        emit_corr_instr(cb, psc, c)
                if cb is not None:
                    finish_corr(cb, psc)

                pending.append(t)
                while pending and sct_by_block[pending[0] // 4] is not None:
                    emit_routing(pending.pop(0))
    nc.finalize()
    return nc


def _host_prep(x, weight, bias):
    """fp16 hi + fp8 lo split, d-major transpose, per-core shards."""
    x = np.asarray(x, dtype=np.float32)
    weight = np.asarray(weight, dtype=np.float32)
    bias = np.asarray(bias, dtype=np.float32)
    f8 = ml_dtypes.float8_e4m3

    wh = weight.astype(np.float16)
    wl = ((weight - wh.astype(np.float32)) * SPLIT_SCALE).astype(np.float16)
    w_packed = np.empty((D, 2 * E), dtype=np.float16)
    w_packed[:, :E] = wh.T
    w_packed[:, E:] = wl.T
    wh8 = np.ascontiguousarray(wh.T).astype(f8)
    bias_rep = np.ascontiguousarray(np.broadcast_to(bias[None, :], (128, E)))
    ident = np.eye(128, dtype=np.float16)

    in_maps = [None] * N_CORES

    def prep_core(c):
        xs = x[c * NSH : (c + 1) * NSH, :]
        xh = xs.astype(np.float16)
        x1 = ((xs - xh.astype(np.float32)) * SPLIT_SCALE).astype(f8)
        in_maps[c] = {
            "xh": np.ascontiguousarray(xh.T),
            "x1": np.ascontiguousarray(x1.T),
            "w": w_packed,
            "wh8": wh8,
            "bias": bias_rep,
            "ident": ident,
        }

    threads = [threading.Thread(target=prep_core, args=(c,)) for c in range(N_CORES)]
    for th in threads:
        th.start()
    for th in threads:
        th.join()
    return in_maps


def kernel(x, weight, bias, _trace=False):
    if "nc" not in _cached:
        _cached["nc"] = _build_nc()
    nc = _cached["nc"]
    in_maps = _host_prep(x, weight, bias)
    res = run_bass_kernel_spmd(
        nc, in_maps, core_ids=list(range(N_CORES)), trace=_trace
    )
    _cached["last_result"] = res
    wts = np.concatenate([r["wts"] for r in res.results], axis=0)
    idx = np.concatenate([r["idx"] for r in res.results], axis=0)
    return wts, idx
